# revision 3
# baseline (speedup 1.0000x reference)
"""Trainium2 Bass kernel for nn_DEACA_attention (dense_transformer).

Strategy (8 cores, data-parallel over batch B=8):

Launch A (per core c = batch c):
  - q convs: load q_row/q_col [T,C], PE-transpose to [C,T], 1x1 conv matmuls,
    BN affine on ACT.
  - k_row/k_col: the reference takes conv2d_bn(...).mean(axis) — mean commutes
    with the 1x1 conv, so reduce first (CCE accumulate-DMA chains at full fp32
    precision), then a single [C,C]@[C,128] conv.
  - v path: the reference's "faithful raw reshape" scrambles (C,H,B,W) into
    (H',W',B*nh,hd).  Algebra reduces everything needed downstream to:
      T_m[e,u]   = sum_{h= m mod 3} v[h,u,e]            (DVE adds)
      M_m        = g_v * (W_v @ T_m)                    (PE + ACT)
      v_row_full[r] = (1/128)*sum_j E_j @ M_{(j+r)%3} + beta_row[r]
      P_m[h,u]   = sum_e Wcol_m[e] v[h,u,e]             (PE transposes + matmuls)
      v_col_full[r] = (1/128)*Perm_{m,r} @ P_m + beta_col[r]
      vm/v_avg   = SE gate from column sums of v_row_full (PE ones-matmul + ACT sigmoid)
  Outputs per core: qr,qc,kr,kc [C,T], v_row_full/v_col_full [3,128,128], v_avg [64,6].

Host exchange: head n needs v_* from batch (n%16)//2 (the scramble crosses
batches) — a small (~400KB/core) numpy reshard between launches.

Launch B (per core c = heads 6c..6c+5): gate k by v_avg, attention row+col
(scores -> softmax -> A@V with PE transpose of exp-scores), head reassembly,
relu, convs pr/pc/p, final transpose to [T,C].
"""

import numpy as np
from contextlib import ExitStack

import concourse.bass as bass
import concourse.bacc as bacc
import concourse.mybir as mybir
import concourse.tile as tile
from concourse.bass_utils import run_bass_kernel_spmd
from concourse.masks import make_identity

F32 = mybir.dt.float32
AF = mybir.ActivationFunctionType
ALU = mybir.AluOpType

B, T, H, W, C = 8, 128, 128, 128, 384
NH, HD = 6, 64
SCALING = float(HD) ** -0.5

_CACHE = {}
_TIMINGS = {}


# ----------------------------------------------------------------------------
# host-side constant prep
# ----------------------------------------------------------------------------

def _prep_consts(p):
    f64 = np.float64
    out = {}
    for nm in ("q_row", "q_col", "k_row", "k_col", "v", "pr", "pc", "p"):
        out["WT_" + nm] = np.ascontiguousarray(p["W_" + nm].T.astype(np.float32))
    out["WT_pw"] = np.ascontiguousarray(p["W_pw"].T.astype(np.float32))

    gv = p["g_v"].astype(f64)
    bv = p["b_v"].astype(f64)
    Wv = p["W_v"].astype(f64)

    wcol = np.zeros((C, 3), f64)
    for j in range(3):
        cs = np.arange(j, C, 3)
        wcol[:, j] = (gv[cs, None] * Wv[cs]).sum(0)
    out["wcol"] = wcol.astype(np.float32)

    cnt = np.array([43.0, 43.0, 42.0], f64)
    brow = np.zeros((3, 128), f64)
    for r in range(3):
        for hp in range(128):
            brow[r, hp] = sum(cnt[(j + r) % 3] * bv[3 * hp + j] for j in range(3)) / 128.0
    out["brow"] = brow.astype(np.float32)

    bsum = np.array([bv[np.arange(j, C, 3)].sum() for j in range(3)], f64)
    bcol = np.zeros((3, 128), f64)
    for r in range(3):
        for wp in range(128):
            bcol[r, wp] = bsum[(3 * wp + r) // 128] / 128.0
    out["bcol"] = bcol.astype(np.float32)

    # E_j^T [c, h'] = 1 iff c == 3h'+j ; stored as 9 blocks [3j+kt][128,128]
    et9 = np.zeros((9, 128, 128), np.float32)
    for j in range(3):
        for cc in range(C):
            if (cc - j) % 3 == 0:
                hp = (cc - j) // 3
                if 0 <= hp < 128:
                    et9[3 * j + cc // 128, cc % 128, hp] = 1.0
    out["et9"] = et9

    # Perm_{m,r}^T [h, w'] = 1 iff (3w'+r)//128 == m and (3w'+r)%128 == h
    pm9 = np.zeros((9, 128, 128), np.float32)
    for m in range(3):
        for r in range(3):
            for wp in range(128):
                if (3 * wp + r) // 128 == m:
                    pm9[3 * m + r, (3 * wp + r) % 128, wp] = 1.0
    out["pm9"] = pm9

    # vec bundle rows (each [C]):
    # 0 g_qr 1 b_qr 2 g_qc 3 b_qc 4 g_kr/128 5 b_kr 6 g_kc/128 7 b_kc 8 g_v
    vecs = np.zeros((9, C), np.float32)
    vecs[0] = p["g_q_row"]; vecs[1] = p["b_q_row"]
    vecs[2] = p["g_q_col"]; vecs[3] = p["b_q_col"]
    vecs[4] = p["g_k_row"] / 128.0; vecs[5] = p["b_k_row"]
    vecs[6] = p["g_k_col"] / 128.0; vecs[7] = p["b_k_col"]
    vecs[8] = p["g_v"]
    out["vecs"] = vecs

    pwv = np.zeros((2, HD), np.float32)
    pwv[0] = p["g_pw"] / 128.0
    pwv[1] = p["b_pw"]
    out["pwv"] = pwv

    # B-launch vec bundle: g_pr b_pr g_pc b_pc g_p b_p
    bvecs = np.stack([p["g_pr"], p["b_pr"], p["g_pc"], p["b_pc"], p["g_p"], p["b_p"]]).astype(np.float32)
    out["bvecs"] = bvecs
    return out


# ----------------------------------------------------------------------------
# launch A builder
# ----------------------------------------------------------------------------

def _vec_col(nc, pool, vecs_d, row, name):
    """Load vec bundle row -> [128, 3] tile (column kt = partitions of c-tile kt)."""
    t = pool.tile([128, 3], F32, tag=name)
    nc.sync.dma_start(t[:], vecs_d[row].rearrange("(kt p) -> p kt", p=128))
    return t


def build_A():
    nc = bacc.Bacc(num_devices=8)
    qrow_d = nc.dram_tensor("qrow", [T, C], F32, kind="ExternalInput")
    qcol_d = nc.dram_tensor("qcol", [T, C], F32, kind="ExternalInput")
    krow_d = nc.dram_tensor("krow", [H, W, C], F32, kind="ExternalInput")
    kcol_d = nc.dram_tensor("kcol", [H, W, C], F32, kind="ExternalInput")
    v_d = nc.dram_tensor("v", [H, W, C], F32, kind="ExternalInput")
    wts_d = {}
    for nm in ("q_row", "q_col", "k_row", "k_col", "v"):
        wts_d[nm] = nc.dram_tensor("wt_" + nm, [C, C], F32, kind="ExternalInput")
    vecs_d = nc.dram_tensor("vecs", [9, C], F32, kind="ExternalInput")
    wcol_d = nc.dram_tensor("wcol", [C, 3], F32, kind="ExternalInput")
    brow_d = nc.dram_tensor("brow", [3, 128], F32, kind="ExternalInput")
    bcol_d = nc.dram_tensor("bcol", [3, 128], F32, kind="ExternalInput")
    et9_d = nc.dram_tensor("et9", [9, 128, 128], F32, kind="ExternalInput")
    pm9_d = nc.dram_tensor("pm9", [9, 128, 128], F32, kind="ExternalInput")
    wpw_d = nc.dram_tensor("wpw", [HD, HD], F32, kind="ExternalInput")
    pwv_d = nc.dram_tensor("pwv", [2, HD], F32, kind="ExternalInput")

    qr_o = nc.dram_tensor("qr_o", [C, T], F32, kind="ExternalOutput")
    qc_o = nc.dram_tensor("qc_o", [C, T], F32, kind="ExternalOutput")
    kr_o = nc.dram_tensor("kr_o", [C, T], F32, kind="ExternalOutput")
    kc_o = nc.dram_tensor("kc_o", [C, T], F32, kind="ExternalOutput")
    vrow_o = nc.dram_tensor("vrow_o", [3, 128, 128], F32, kind="ExternalOutput")
    vcol_o = nc.dram_tensor("vcol_o", [3, 128, 128], F32, kind="ExternalOutput")
    vavg_o = nc.dram_tensor("vavg_o", [HD, 6], F32, kind="ExternalOutput")

    with tile.TileContext(nc) as tc, ExitStack() as ctx:
        consts = ctx.enter_context(tc.tile_pool(name="consts", bufs=1))
        sb = ctx.enter_context(tc.tile_pool(name="sb", bufs=1))
        stream = ctx.enter_context(tc.tile_pool(name="stream", bufs=3))
        vtp = ctx.enter_context(tc.tile_pool(name="vtp", bufs=4))
        ps = ctx.enter_context(tc.tile_pool(name="ps", bufs=1, space="PSUM"))
        psc = ctx.enter_context(tc.tile_pool(name="psc", bufs=2, space="PSUM"))
        pst = ctx.enter_context(tc.tile_pool(name="pst", bufs=4, space="PSUM"))

        ident = consts.tile([128, 128], F32)
        make_identity(nc, ident[:])
        ones1 = consts.tile([128, 1], F32)
        nc.vector.memset(ones1[:], 1.0)

        # weights: per name, 3 tiles [128, 384] (rows e-block kt)
        wt_sb = {}
        for nm in ("q_row", "q_col", "k_row", "k_col", "v"):
            tiles = []
            for kt in range(3):
                wtile = consts.tile([128, C], F32, tag=f"w_{nm}_{kt}")
                nc.sync.dma_start(wtile[:], wts_d[nm][128 * kt : 128 * kt + 128, :])
                tiles.append(wtile)
            wt_sb[nm] = tiles

        wcol9 = consts.tile([128, 9], F32)
        nc.sync.dma_start(wcol9[:], wcol_d[:].rearrange("(kt p) m -> p kt m", p=128))
        brow_t = consts.tile([128, 3], F32)
        nc.sync.dma_start(brow_t[:], brow_d[:].rearrange("r hp -> hp r"))
        bcol_t = consts.tile([128, 3], F32)
        nc.sync.dma_start(bcol_t[:], bcol_d[:].rearrange("r wp -> wp r"))
        et9_t = consts.tile([128, 9 * 128], F32)
        nc.sync.dma_start(et9_t[:], et9_d[:].rearrange("b p h -> p b h"))
        pm9_t = consts.tile([128, 9 * 128], F32)
        nc.sync.dma_start(pm9_t[:], pm9_d[:].rearrange("b p h -> p b h"))
        wpw_t = consts.tile([HD, HD], F32)
        nc.sync.dma_start(wpw_t[:], wpw_d[:])
        pwv_t = consts.tile([HD, 2], F32)
        nc.sync.dma_start(pwv_t[:], pwv_d[:].rearrange("r d -> d r"))

        gcols = {}
        for i, nm in enumerate(
            ("g_qr", "b_qr", "g_qc", "b_qc", "g_kr", "b_kr", "g_kc", "b_kc", "g_v")
        ):
            gcols[nm] = _vec_col(nc, consts, vecs_d, i, nm)

        # ------------- q paths -------------
        def conv_and_out(wname, rhs_sb, g_col, b_col, out_d, scale_from_g=True):
            for mt in range(3):
                pc = psc.tile([128, 128], F32, tag="conv")
                for kt in range(3):
                    nc.tensor.matmul(
                        pc[:],
                        wt_sb[wname][kt][:, 128 * mt : 128 * mt + 128],
                        rhs_sb[:, 128 * kt : 128 * kt + 128],
                        start=(kt == 0),
                        stop=(kt == 2),
                    )
                o = sb.tile([128, 128], F32, tag="convo")
                nc.scalar.activation(
                    o[:], pc[:], AF.Identity,
                    bias=b_col[:, mt : mt + 1], scale=g_col[:, mt : mt + 1],
                )
                nc.sync.dma_start(out_d[128 * mt : 128 * mt + 128, :], o[:])

        for (q_d, wname, gn, bn, out_d) in (
            (qrow_d, "q_row", "g_qr", "b_qr", qr_o),
            (qcol_d, "q_col", "g_qc", "b_qc", qc_o),
        ):
            qt = sb.tile([128, C], F32, tag="qt_" + wname)
            nc.sync.dma_start(qt[:], q_d[:])
            qx = sb.tile([128, C], F32, tag="qx_" + wname)
            for ec in range(3):
                ptr = pst.tile([128, 128], F32, tag="tr")
                nc.tensor.transpose(ptr[:], qt[:, 128 * ec : 128 * ec + 128], ident[:])
                nc.scalar.copy(qx[:, 128 * ec : 128 * ec + 128], ptr[:])
            conv_and_out(wname, qx[:], gcols[gn], gcols[bn], out_d)

        # ------------- k reductions via CCE accumulate-DMA -------------
        # k_row: sum over w -> acc tiles [h, (4 w-slices, e)]
        # k_col: sum over h -> acc tiles [w, (4 h-slices, e)]
        NCH = 4       # parallel chains
        PERD = 4      # reduced slices per DMA
        def cce_reduce(src_d, which, name):
            accs = []
            for ch in range(NCH):
                acc = sb.tile([128, PERD * C], F32, tag=f"{name}_acc{ch}")
                accs.append(acc)
            steps = 128 // (NCH * PERD)  # 8
            for ch in range(NCH):
                for s in range(steps):
                    i0 = (ch * steps + s) * PERD
                    if which == "w":   # reduce over w: tiles [h, e] per w
                        src = src_d[:, i0 : i0 + PERD, :]
                    else:              # reduce over h: tiles [w, e] per h
                        src = src_d[i0 : i0 + PERD, :, :].rearrange("h w e -> w h e")
                    if s == 0:
                        nc.gpsimd.dma_start(accs[ch][:], src)
                    else:
                        nc.gpsimd.dma_start(accs[ch][:], src, accum_op=ALU.add)
            # fold: 16 [128, C] partials -> one
            xsum = sb.tile([128, C], F32, tag=f"{name}_sum")
            parts = [accs[ch][:, C * i : C * i + C] for ch in range(NCH) for i in range(PERD)]
            t1 = []
            for i in range(8):
                tt = sb.tile([128, C], F32, tag=f"{name}_f{i % 4}")
                nc.vector.tensor_add(tt[:], parts[2 * i], parts[2 * i + 1])
                t1.append(tt)
            t2 = []
            for i in range(4):
                tt = sb.tile([128, C], F32, tag=f"{name}_g{i % 2}")
                nc.vector.tensor_add(tt[:], t1[2 * i][:], t1[2 * i + 1][:])
                t2.append(tt)
            t3a = sb.tile([128, C], F32, tag=f"{name}_h0")
            nc.vector.tensor_add(t3a[:], t2[0][:], t2[1][:])
            t3b = sb.tile([128, C], F32, tag=f"{name}_h1")
            nc.vector.tensor_add(t3b[:], t2[2][:], t2[3][:])
            nc.vector.tensor_add(xsum[:], t3a[:], t3b[:])
            return xsum

        for (src_d, which, name, wname, gn, bn, out_d) in (
            (krow_d, "w", "kr", "k_row", "g_kr", "b_kr", kr_o),
            (kcol_d, "h", "kc", "k_col", "g_kc", "b_kc", kc_o),
        ):
            xsum = cce_reduce(src_d, which, name)  # [128, C] ([h,e] or [w,e])
            xt = sb.tile([128, C], F32, tag=name + "_xt")
            for ec in range(3):
                ptr = pst.tile([128, 128], F32, tag="tr")
                nc.tensor.transpose(ptr[:], xsum[:, 128 * ec : 128 * ec + 128], ident[:])
                nc.scalar.copy(xt[:, 128 * ec : 128 * ec + 128], ptr[:])
            conv_and_out(wname, xt[:], gcols[gn], gcols[bn], out_d)

        # ------------- v stream: T sums + P -------------
        tacc = []
        for m in range(3):
            ta = sb.tile([128, C], F32, tag=f"tacc{m}")
            tacc.append(ta)
        ptw = ps.tile([128, C], F32, tag="ptw")  # PT accumulator [u, (h, m)]

        HC = 8
        for h0 in range(0, H, HC):
            vch = stream.tile([128, HC * C], F32, tag="vch")
            nc.sync.dma_start(
                vch[:], v_d[h0 : h0 + HC, :, :].rearrange("h w e -> w h e")
            )
            for hl in range(HC):
                h = h0 + hl
                m = h % 3
                if h < 3:
                    nc.vector.tensor_copy(tacc[m][:], vch[:, C * hl : C * hl + C])
                else:
                    nc.vector.tensor_add(
                        tacc[m][:], tacc[m][:], vch[:, C * hl : C * hl + C]
                    )
                vt = vtp.tile([128, C], F32, tag="vt")
                for ec in range(3):
                    ptr = pst.tile([128, 128], F32, tag="tr")
                    nc.tensor.transpose(
                        ptr[:],
                        vch[:, C * hl + 128 * ec : C * hl + 128 * ec + 128],
                        ident[:],
                    )
                    if ec == 1:
                        nc.vector.tensor_copy(vt[:, 128 * ec : 128 * ec + 128], ptr[:])
                    else:
                        nc.scalar.copy(vt[:, 128 * ec : 128 * ec + 128], ptr[:])
                for ec in range(3):
                    nc.tensor.matmul(
                        ptw[:, 3 * h : 3 * h + 3],
                        vt[:, 128 * ec : 128 * ec + 128],
                        wcol9[:, 3 * ec : 3 * ec + 3],
                        start=(ec == 0),
                        stop=(ec == 2),
                    )

        # PT -> P_m [h, u]
        p_sb = []
        for m in range(3):
            ptm = sb.tile([128, 128], F32, tag=f"ptm{m}")
            nc.vector.tensor_copy(
                ptm[:], ptw[:].rearrange("p (h m) -> p m h", m=3)[:, m]
            )
            ptr = pst.tile([128, 128], F32, tag="tr")
            nc.tensor.transpose(ptr[:], ptm[:], ident[:])
            pm = sb.tile([128, 128], F32, tag=f"pm{m}")
            nc.scalar.copy(pm[:], ptr[:])
            p_sb.append(pm)

        # T transposes -> Tt_m [e(kt-major), u]
        tt_sb = []
        for m in range(3):
            ttm = sb.tile([128, C], F32, tag=f"ttm{m}")
            for ec in range(3):
                ptr = pst.tile([128, 128], F32, tag="tr")
                nc.tensor.transpose(ptr[:], tacc[m][:, 128 * ec : 128 * ec + 128], ident[:])
                nc.scalar.copy(ttm[:, 128 * ec : 128 * ec + 128], ptr[:])
            tt_sb.append(ttm)

        # M_m = g_v * (W_v @ T_m), tiles [128, 384] (mt-major)
        m_sb = []
        for m in range(3):
            mm = sb.tile([128, C], F32, tag=f"msb{m}")
            for mt in range(3):
                pc = psc.tile([128, 128], F32, tag="conv")
                for kt in range(3):
                    nc.tensor.matmul(
                        pc[:],
                        wt_sb["v"][kt][:, 128 * mt : 128 * mt + 128],
                        tt_sb[m][:, 128 * kt : 128 * kt + 128],
                        start=(kt == 0),
                        stop=(kt == 2),
                    )
                nc.scalar.activation(
                    mm[:, 128 * mt : 128 * mt + 128], pc[:], AF.Copy,
                    bias=0.0, scale=gcols["g_v"][:, mt : mt + 1],
                )
            m_sb.append(mm)

        # v_row_full[r] = (1/128) sum_j E_j @ M_{(j+r)%3} + beta_row[r]
        vrow_sb = []
        for r in range(3):
            pv = psc.tile([128, 128], F32, tag="conv")
            first = True
            for j in range(3):
                msrc = m_sb[(j + r) % 3]
                for kt in range(3):
                    nc.tensor.matmul(
                        pv[:],
                        et9_t[:, 128 * (3 * j + kt) : 128 * (3 * j + kt) + 128],
                        msrc[:, 128 * kt : 128 * kt + 128],
                        start=first,
                        stop=(j == 2 and kt == 2),
                    )
                    first = False
            o = sb.tile([128, 128], F32, tag=f"vrow{r}")
            nc.scalar.activation(
                o[:], pv[:], AF.Identity,
                bias=brow_t[:, r : r + 1], scale=1.0 / 128.0,
            )
            vrow_sb.append(o)
            nc.sync.dma_start(vrow_o[r], o[:])

        # v_col_full[r] = (1/128) sum_m Perm_{m,r} @ P_m + beta_col[r]
        for r in range(3):
            pv = psc.tile([128, 128], F32, tag="conv")
            for m in range(3):
                nc.tensor.matmul(
                    pv[:],
                    pm9_t[:, 128 * (3 * m + r) : 128 * (3 * m + r) + 128],
                    p_sb[m][:],
                    start=(m == 0),
                    stop=(m == 2),
                )
            o = sb.tile([128, 128], F32, tag=f"vcol{r}")
            nc.scalar.activation(
                o[:], pv[:], AF.Identity,
                bias=bcol_t[:, r : r + 1], scale=1.0 / 128.0,
            )
            nc.sync.dma_start(vcol_o[r], o[:])

        # vm / v_avg
        vm6 = sb.tile([HD, 6], F32)
        for r in range(3):
            pvm = ps.tile([128, 1], F32, tag="vmp")
            nc.tensor.matmul(pvm[:], vrow_sb[r][:], ones1[:], start=True, stop=True)
            for half in range(2):
                slot = 2 * r + half
                nc.vector.tensor_copy(
                    vm6[:, slot : slot + 1], pvm[64 * half : 64 * half + 64, :]
                )
        pavg = ps.tile([HD, 6], F32, tag="vmp")
        nc.tensor.matmul(pavg[:], wpw_t[:], vm6[:], start=True, stop=True)
        vavg_sb = sb.tile([HD, 6], F32)
        nc.scalar.activation(
            vavg_sb[:], pavg[:], AF.Sigmoid,
            bias=pwv_t[:, 1:2], scale=pwv_t[:, 0:1],
        )
        nc.sync.dma_start(vavg_o[:], vavg_sb[:])

    nc.compile()
    return nc


# ----------------------------------------------------------------------------
# launch B builder
# ----------------------------------------------------------------------------

def build_B():
    nc = bacc.Bacc(num_devices=8)
    qr_d = nc.dram_tensor("qr", [C, T], F32, kind="ExternalInput")
    qc_d = nc.dram_tensor("qc", [C, T], F32, kind="ExternalInput")
    kr_d = nc.dram_tensor("kr", [C, T], F32, kind="ExternalInput")
    kc_d = nc.dram_tensor("kc", [C, T], F32, kind="ExternalInput")
    vrow_d = nc.dram_tensor("vrow6", [6, 128, HD], F32, kind="ExternalInput")
    vcol_d = nc.dram_tensor("vcol6", [6, 128, HD], F32, kind="ExternalInput")
    vavg_d = nc.dram_tensor("vavg6", [HD, 6], F32, kind="ExternalInput")
    wts_d = {}
    for nm in ("pr", "pc", "p"):
        wts_d[nm] = nc.dram_tensor("wt_" + nm, [C, C], F32, kind="ExternalInput")
    bvecs_d = nc.dram_tensor("bvecs", [6, C], F32, kind="ExternalInput")
    out_d = nc.dram_tensor("out_bt", [T, C], F32, kind="ExternalOutput")

    with tile.TileContext(nc) as tc, ExitStack() as ctx:
        consts = ctx.enter_context(tc.tile_pool(name="consts", bufs=1))
        sb = ctx.enter_context(tc.tile_pool(name="sb", bufs=1))
        wk = ctx.enter_context(tc.tile_pool(name="wk", bufs=2))
        ps = ctx.enter_context(tc.tile_pool(name="ps", bufs=2, space="PSUM"))
        ps1 = ctx.enter_context(tc.tile_pool(name="ps1", bufs=1, space="PSUM"))

        ident = consts.tile([128, 128], F32)
        make_identity(nc, ident[:])

        wt_sb = {}
        for nm in ("pr", "pc", "p"):
            tiles = []
            for kt in range(3):
                wtile = consts.tile([128, C], F32, tag=f"w_{nm}_{kt}")
                nc.sync.dma_start(wtile[:], wts_d[nm][128 * kt : 128 * kt + 128, :])
                tiles.append(wtile)
            wt_sb[nm] = tiles

        gcols = {}
        for i, nm in enumerate(("g_pr", "b_pr", "g_pc", "b_pc", "g_p", "b_p")):
            t = consts.tile([128, 3], F32, tag=nm)
            nc.sync.dma_start(t[:], bvecs_d[i].rearrange("(kt p) -> p kt", p=128))
            gcols[nm] = t

        # inputs: [c-major] tiles [128, 384] (free = ct*128 + t)
        def load_ct(d, name):
            t = consts.tile([128, C], F32, tag=name)
            nc.sync.dma_start(t[:], d[:].rearrange("(ct p) t -> p ct t", p=128))
            return t

        qr_t = load_ct(qr_d, "qr_t")
        qc_t = load_ct(qc_d, "qc_t")
        kr_t = load_ct(kr_d, "kr_t")
        kc_t = load_ct(kc_d, "kc_t")

        vrow_t = consts.tile([128, 6 * HD], F32)
        nc.sync.dma_start(vrow_t[:], vrow_d[:].rearrange("k s d -> s k d"))
        vcol_t = consts.tile([128, 6 * HD], F32)
        nc.sync.dma_start(vcol_t[:], vcol_d[:].rearrange("k s d -> s k d"))
        vavg_t = consts.tile([HD, 6], F32)
        nc.sync.dma_start(vavg_t[:], vavg_d[:])

        vavg128 = sb.tile([128, 3], F32)
        for k in range(6):
            nc.vector.tensor_copy(
                vavg128[64 * (k % 2) : 64 * (k % 2) + 64, k // 2 : k // 2 + 1],
                vavg_t[:, k : k + 1],
            )

        # gate k
        krg = sb.tile([128, C], F32)
        kcg = sb.tile([128, C], F32)
        for kt in range(3):
            nc.vector.tensor_scalar(
                krg[:, 128 * kt : 128 * kt + 128], kr_t[:, 128 * kt : 128 * kt + 128],
                vavg128[:, kt : kt + 1], None, op0=ALU.mult,
            )
            nc.vector.tensor_scalar(
                kcg[:, 128 * kt : 128 * kt + 128], kc_t[:, 128 * kt : 128 * kt + 128],
                vavg128[:, kt : kt + 1], None, op0=ALU.mult,
            )

        xr = sb.tile([128, C], F32)
        xc = sb.tile([128, C], F32)

        for k in range(6):
            po = 64 * (k % 2)
            fo = 128 * (k // 2)
            for (qt, kg, vsl, xout) in (
                (qr_t, krg, vrow_t, xr),
                (qc_t, kcg, vcol_t, xc),
            ):
                psc = ps.tile([128, 128], F32, tag="sc")
                nc.tensor.matmul(
                    psc[:],
                    qt[po : po + 64, fo : fo + 128],
                    kg[po : po + 64, fo : fo + 128],
                    start=True, stop=True,
                )
                mx = wk.tile([128, 1], F32, tag="mx")
                nc.vector.reduce_max(out=mx[:], in_=psc[:], axis=mybir.AxisListType.X)
                mxs = wk.tile([128, 1], F32, tag="mxs")
                nc.vector.tensor_scalar_mul(mxs[:], mx[:], -SCALING)
                et = wk.tile([128, 128], F32, tag="et")
                nc.scalar.activation(
                    et[:], psc[:], AF.Exp, bias=mxs[:], scale=SCALING,
                )
                sm = wk.tile([128, 1], F32, tag="sm")
                nc.vector.reduce_sum(out=sm[:], in_=et[:], axis=mybir.AxisListType.X)
                rc = wk.tile([128, 1], F32, tag="rc")
                nc.vector.reciprocal(rc[:], sm[:])
                ptr = ps.tile([128, 128], F32, tag="tr")
                nc.tensor.transpose(ptr[:], et[:], ident[:])
                ets = wk.tile([128, 128], F32, tag="ets")
                nc.scalar.copy(ets[:], ptr[:])
                pxx = ps1.tile([128, HD], F32, tag="xx")
                nc.tensor.matmul(
                    pxx[:], ets[:], vsl[:, HD * k : HD * k + HD], start=True, stop=True
                )
                xxn = wk.tile([128, HD], F32, tag="xxn")
                nc.vector.tensor_scalar(
                    xxn[:], pxx[:], rc[:], None, op0=ALU.mult,
                )
                pxt = ps1.tile([64, 128], F32, tag="xt")
                nc.tensor.transpose(pxt[:], xxn[:], ident[:])
                nc.scalar.activation(
                    xout[po : po + 64, fo : fo + 128], pxt[:], AF.Relu,
                )

        # convs
        def conv(wname, rhs, g, b, out_cb):
            for mt in range(3):
                pc = ps1.tile([128, 128], F32, tag="cv")
                for kt in range(3):
                    nc.tensor.matmul(
                        pc[:],
                        wt_sb[wname][kt][:, 128 * mt : 128 * mt + 128],
                        rhs[:, 128 * kt : 128 * kt + 128],
                        start=(kt == 0), stop=(kt == 2),
                    )
                out_cb(mt, pc, g, b)

        y1 = sb.tile([128, C], F32)

        def y1_cb(mt, pc, g, b):
            nc.scalar.activation(
                y1[:, 128 * mt : 128 * mt + 128], pc[:], AF.Identity,
                bias=b[:, mt : mt + 1], scale=g[:, mt : mt + 1],
            )

        conv("pr", xr[:], gcols["g_pr"], gcols["b_pr"], y1_cb)

        rl = sb.tile([128, C], F32)

        def y2_cb(mt, pc, g, b):
            y2t = wk.tile([128, 128], F32, tag="y2t")
            nc.scalar.activation(
                y2t[:], pc[:], AF.Identity,
                bias=b[:, mt : mt + 1], scale=g[:, mt : mt + 1],
            )
            st = wk.tile([128, 128], F32, tag="st")
            nc.vector.tensor_add(st[:], y1[:, 128 * mt : 128 * mt + 128], y2t[:])
            nc.scalar.activation(rl[:, 128 * mt : 128 * mt + 128], st[:], AF.Relu)

        conv("pc", xc[:], gcols["g_pc"], gcols["b_pc"], y2_cb)

        out_sb = sb.tile([128, C], F32)

        def y_cb(mt, pc, g, b):
            yt = wk.tile([128, 128], F32, tag="yt")
            nc.scalar.activation(
                yt[:], pc[:], AF.Identity,
                bias=b[:, mt : mt + 1], scale=g[:, mt : mt + 1],
            )
            ptr = ps1.tile([128, 128], F32, tag="otr")
            nc.tensor.transpose(ptr[:], yt[:], ident[:])
            nc.scalar.copy(out_sb[:, 128 * mt : 128 * mt + 128], ptr[:])

        conv("p", rl[:], gcols["g_p"], gcols["b_p"], y_cb)
        nc.sync.dma_start(out_d[:], out_sb[:])

    nc.compile()
    return nc


# ----------------------------------------------------------------------------
# host orchestration
# ----------------------------------------------------------------------------

def kernel(q_row, q_col, k_row, k_col, v, params):
    q_row = np.ascontiguousarray(q_row, dtype=np.float32)
    q_col = np.ascontiguousarray(q_col, dtype=np.float32)
    k_row = np.ascontiguousarray(k_row, dtype=np.float32)
    k_col = np.ascontiguousarray(k_col, dtype=np.float32)
    v = np.ascontiguousarray(v, dtype=np.float32)
    p = {k: np.asarray(val, dtype=np.float32) for k, val in params.items()}
    cst = _prep_consts(p)

    if "A" not in _CACHE:
        _CACHE["A"] = build_A()
    if "B" not in _CACHE:
        _CACHE["B"] = build_B()
    ncA, ncB = _CACHE["A"], _CACHE["B"]

    a_maps = []
    for c in range(B):
        m = {
            "qrow": q_row[c], "qcol": q_col[c],
            "krow": k_row[c], "kcol": k_col[c], "v": v[c],
            "wt_q_row": cst["WT_q_row"], "wt_q_col": cst["WT_q_col"],
            "wt_k_row": cst["WT_k_row"], "wt_k_col": cst["WT_k_col"],
            "wt_v": cst["WT_v"],
            "vecs": cst["vecs"], "wcol": cst["wcol"],
            "brow": cst["brow"], "bcol": cst["bcol"],
            "et9": cst["et9"], "pm9": cst["pm9"],
            "wpw": cst["WT_pw"], "pwv": cst["pwv"],
        }
        a_maps.append(m)
    import time as _time
    _t0 = _time.time()
    resA = run_bass_kernel_spmd(ncA, a_maps, core_ids=list(range(B))).results
    _TIMINGS["A_s"] = _time.time() - _t0

    b_maps = []
    for c in range(B):
        vrow6 = np.zeros((6, 128, HD), np.float32)
        vcol6 = np.zeros((6, 128, HD), np.float32)
        vavg6 = np.zeros((HD, 6), np.float32)
        for k in range(6):
            n = 6 * c + k
            owner = (n % 16) // 2
            rn = n // 16
            half = n % 2
            slot = 2 * rn + half
            wo = 64 * half
            vrow6[k] = resA[owner]["vrow_o"][rn][:, wo : wo + HD]
            vcol6[k] = resA[owner]["vcol_o"][rn][:, wo : wo + HD]
            vavg6[:, k] = resA[owner]["vavg_o"][:, slot]
        m = {
            "qr": resA[c]["qr_o"], "qc": resA[c]["qc_o"],
            "kr": resA[c]["kr_o"], "kc": resA[c]["kc_o"],
            "vrow6": vrow6, "vcol6": vcol6, "vavg6": vavg6,
            "wt_pr": cst["WT_pr"], "wt_pc": cst["WT_pc"], "wt_p": cst["WT_p"],
            "bvecs": cst["bvecs"],
        }
        b_maps.append(m)
    _t0 = _time.time()
    resB = run_bass_kernel_spmd(ncB, b_maps, core_ids=list(range(B))).results
    _TIMINGS["B_s"] = _time.time() - _t0
    _TIMINGS["exec_ns"] = int((_TIMINGS["A_s"] + _TIMINGS["B_s"]) * 1e9)

    out = np.zeros((T, B, C), np.float32)
    for c in range(B):
        out[:, c, :] = resB[c]["out_bt"]
    return out
